# revision 41
# baseline (speedup 1.0000x reference)
"""EnhancedGCN on 8 Trainium2 NeuronCores (Bass/Tile, SPMD).

Strategy: 1D node partition (6250 nodes/core, padded to 6272). Small weights
replicated. Per propagation step: each core computes xws = dis * (h @ conv_w.T)
for its nodes, AllGathers the bf16 table (4 collectives writing slices of two
int16-addressable super-tables, pipelined behind the xws windows), then
gathers source rows per edge (dma_gather over 4 SWDGE queues), reduces them
into per-target sums with 0/1 selection-matrix matmuls accumulating in PSUM
(self-loops enter as an identity-matmul block), and applies the pointwise
epilogue (degree norm, root/relu term, residual+LN between steps).
Edge weights ew = dis[t]*dis[s] are separable: they fold into a pre-scale of
the table (dis[s]) and a post-scale of the message sum (dis[t]).
Host-side work is limited to graph-structure prep (sorting edges into
target windows, block padding, int16 index streams) and weight transposes.
"""
import sys

sys.path.insert(0, "/opt/trn_rl_repo")

import numpy as np
import ml_dtypes

import concourse.bass as bass
import concourse.bacc as bacc
import concourse.tile as tile
import concourse.mybir as mybir
from concourse.bass_utils import run_bass_kernel_spmd
from concourse.masks import make_identity

BF16 = ml_dtypes.bfloat16
N, IN, H = 50000, 256, 128
NCORES = 8
NPC = N // NCORES  # 6250
NW = (NPC + 127) // 128  # 49
PADN = NW * 128  # 6272
LN_EPS = 1e-5
NGRP = (NW + 3) // 4  # 13 groups of 4 windows

# Two gather super-streams, one AllGather each (Shared-output collectives
# require a single writing instruction per Shared tensor).
NSUP = 2
SUP_W = [24, 25]  # windows per super
SUP_W0 = [0, 24, 49]
SUP_SZ = [w * 128 for w in SUP_W]
# table row layout per super: [8 ranks x sup_sz]
TBL_ROWS = [8 * SUP_SZ[0], 8 * SUP_SZ[1]]

F32 = mybir.dt.float32
BF = mybir.dt.bfloat16
I16 = mybir.dt.int16
AX = mybir.AluOpType
AF = mybir.ActivationFunctionType


def _bcast_mid(ap, n):
    """[128, F] AP -> [128, n, F] with stride-0 middle dim."""
    a = ap.copy()
    a.ap = [a.ap[0], [0, n]] + a.ap[1:]
    return a


def _r3(ap, f):
    return ap.rearrange("p (w f) -> p w f", f=f)


def _wrap_idx(idx):
    """flat idx [n] (n % 16 == 0) -> [128, n/16] int16 wrapped + replicated."""
    n = len(idx)
    t = idx.reshape(n // 16, 16).T.astype(np.int16)
    return np.tile(t, (8, 1))


def _prep_graph(row, col):
    """Graph-structure-only preprocessing (row/col ints)."""
    deg = np.bincount(row, minlength=N).astype(np.float64) + 1.0
    dis_f = 1.0 / np.sqrt(deg)
    dinv_f = 1.0 / deg

    core = row // NPC
    src_core = col // NPC
    src_off = col % NPC
    src_w = src_off >> 7
    src_sup = (src_w >= SUP_W0[1]).astype(np.int64)
    csz = np.asarray(SUP_SZ)[src_sup]
    w0 = np.asarray(SUP_W0)[src_sup] * 128
    src_idx = src_core * csz + (src_off - w0)

    per_core = []
    counts = np.zeros((NCORES, NW, NSUP), np.int64)
    for k in range(NCORES):
        m = core == k
        tgt = (row[m] - k * NPC).astype(np.int64)
        sidx = src_idx[m]
        ssup = src_sup[m]
        w = tgt >> 7
        order = np.argsort(w, kind="stable")
        tgt, sidx, ssup, w = tgt[order], sidx[order], ssup[order], w[order]
        ents = []
        bounds = np.searchsorted(w, np.arange(NW + 1))
        for wi in range(NW):
            sl = slice(bounds[wi], bounds[wi + 1])
            s_w, t_w, u_w = sidx[sl], tgt[sl] - (wi << 7), ssup[sl]
            by_sup = []
            for s in range(NSUP):
                mm = u_w == s
                # NB: do NOT sort by source here — sorted in-flight gather
                # descriptors concentrate on few DRAM channels and measured
                # 1.7x slower than the random edge order.
                by_sup.append((s_w[mm], t_w[mm]))
                counts[k, wi, s] = int(mm.sum())
            ents.append(by_sup)
        per_core.append(ents)

    B = np.ceil(counts.max(axis=0) / 128).astype(np.int64)  # [NW, NSUP]
    NBLK = B.sum(axis=0).astype(np.int64)  # per super
    gpos = np.zeros((NSUP, NGRP + 1), np.int64)
    for s in range(NSUP):
        pref = np.concatenate([[0], np.cumsum(B[:, s])])
        for g in range(NGRP + 1):
            gpos[s, g] = pref[min(g * 4, NW)]

    # padding descriptors read random rows (masked by -1 tlocs): random spreads
    # the dead reads across DRAM channels like the real traffic
    rng = np.random.default_rng(12345)
    idx_streams = [np.empty((NCORES, int(NBLK[s]) * 128), np.int64) for s in range(NSUP)]
    for s in range(NSUP):
        idx_streams[s][:] = rng.integers(0, TBL_ROWS[s], idx_streams[s].shape)
    tlocs = [np.full((NCORES, 128, int(NBLK[s])), -1.0, np.float32) for s in range(NSUP)]

    for k in range(NCORES):
        pos = [0] * NSUP
        for wi in range(NW):
            for s in range(NSUP):
                s_w, t_w = per_core[k][wi][s]
                n = len(s_w)
                p = pos[s]
                idx_streams[s][k, p * 128 : p * 128 + n] = s_w
                j = np.arange(n)
                tlocs[s][k, j % 128, p + j // 128] = t_w
                pos[s] += int(B[wi, s])

    # wrap idx per half-call segment (aligned to group boundaries, split in two)
    idx_w = [None] * NSUP
    for s in range(NSUP):
        per_core_w = [[] for _ in range(NCORES)]
        for g in range(NGRP):
            b0, b1 = int(gpos[s, g]), int(gpos[s, g + 1])
            mid = b0 + (b1 - b0 + 1) // 2
            for (h0_, h1_) in ((b0, mid), (mid, b1)):
                if h1_ > h0_:
                    for k in range(NCORES):
                        per_core_w[k].append(
                            _wrap_idx(idx_streams[s][k, h0_ * 128 : h1_ * 128])
                        )
        idx_w[s] = np.stack([np.concatenate(x, axis=1) for x in per_core_w])

    dis_cols = np.zeros((NCORES, 128, NW), np.float32)
    dinv_cols = np.ones((NCORES, 128, NW), np.float32)
    for k in range(NCORES):
        v = np.zeros(PADN, np.float64)
        v[:NPC] = dis_f[k * NPC : (k + 1) * NPC]
        dis_cols[k] = v.reshape(NW, 128).T
        u = np.ones(PADN, np.float64)
        u[:NPC] = dinv_f[k * NPC : (k + 1) * NPC]
        dinv_cols[k] = u.reshape(NW, 128).T

    return dict(
        B=B,
        NBLK=NBLK,
        gpos=gpos,
        idx_w=idx_w,
        tlocs=[t.astype(BF16) for t in tlocs],
        dis_cols=dis_cols,
        dinv_cols=dinv_cols,
    )


def _build(B, gpos, NBLK):
    nc = bacc.Bacc("TRN2", target_bir_lowering=False, debug=False, num_swdge_queues=4)

    ift = nc.dram_tensor("ift", [IN, PADN], BF, kind="ExternalInput")
    lin_wT = nc.dram_tensor("lin_wT", [IN, H], BF, kind="ExternalInput")
    lwc_in = nc.dram_tensor("lwc", [IN, H], BF, kind="ExternalInput")
    conv_wT = nc.dram_tensor("conv_wT", [H, H], BF, kind="ExternalInput")
    consts = nc.dram_tensor("consts", [128, 5 * H], F32, kind="ExternalInput")
    iota_in = nc.dram_tensor("iota", [128, 128], BF, kind="ExternalInput")
    discols = nc.dram_tensor("discols", [128, NW], F32, kind="ExternalInput")
    dinvcols = nc.dram_tensor("dinvcols", [128, NW], F32, kind="ExternalInput")
    idx_t = [
        nc.dram_tensor(f"idx{s}", [128, int(NBLK[s]) * 8], I16, kind="ExternalInput")
        for s in range(NSUP)
    ]
    tloc_t = [
        nc.dram_tensor(f"tloc{s}", [128, int(NBLK[s])], BF, kind="ExternalInput")
        for s in range(NSUP)
    ]
    out_ext = nc.dram_tensor("out", [PADN, H], BF, kind="ExternalOutput")

    def ws(w):
        return slice(w * 128, (w + 1) * 128)

    # per-(stream, group) half-call boundaries + column offset into wrapped idx
    halves = {}
    for s in range(NSUP):
        off = 0
        for g in range(NGRP):
            b0, b1 = int(gpos[s, g]), int(gpos[s, g + 1])
            mid = b0 + (b1 - b0 + 1) // 2
            hs = []
            for (h0_, h1_) in ((b0, mid), (mid, b1)):
                hs.append((h0_, h1_, off))
                off += (h1_ - h0_) * 8
            halves[(s, g)] = hs
    gmax = max(h1 - h0 for v in halves.values() for (h0, h1, _) in v)
    wgmax = int(
        max(sum(int(gpos[s, g + 1] - gpos[s, g]) for s in range(NSUP)) for g in range(NGRP))
    )

    with tile.TileContext(nc) as tc:
        with (
            tc.tile_pool(name="const", bufs=1) as cpool,
            tc.tile_pool(name="state", bufs=1) as spool,
            tc.tile_pool(name="iftp", bufs=12) as ipool,
            tc.tile_pool(name="ht", bufs=12) as hpool,
            tc.tile_pool(name="gath", bufs=14) as gpool,
            tc.tile_pool(name="wp", bufs=3) as wpool,
            tc.tile_pool(name="tmp", bufs=1) as tpool,
            tc.tile_pool(name="psA", bufs=4, space="PSUM") as psA,
            tc.tile_pool(name="psM", bufs=3, space="PSUM") as psM,
            tc.tile_pool(name="dram", bufs=1, space="DRAM") as dpool,
        ):
            identf = cpool.tile([128, 128], F32)
            make_identity(nc, identf[:])
            identb = cpool.tile([128, 128], BF)
            nc.vector.tensor_copy(out=identb[:], in_=identf[:])
            cst = cpool.tile([128, 5 * H], F32)
            nc.sync.dma_start(out=cst[:], in_=consts[:])
            linb, rootr, convbr, g1r, b1r = (cst[:, i * H : (i + 1) * H] for i in range(5))
            # bf16 copies for ops whose data operands are bf16 (avoid
            # mixed-input-dtype DVE ops)
            cstb = cpool.tile([128, 5 * H], BF)
            nc.vector.tensor_copy(out=cstb[:], in_=cst[:])
            _, rootb, _, g1b, b1b = (cstb[:, i * H : (i + 1) * H] for i in range(5))
            iot = cpool.tile([128, 128], BF)
            nc.sync.dma_start(out=iot[:], in_=iota_in[:])
            cw = cpool.tile([128, H], BF)
            nc.sync.dma_start(out=cw[:], in_=conv_wT[:])
            lw0 = cpool.tile([128, H], BF)
            nc.sync.dma_start(out=lw0[:], in_=lin_wT[0:128, :])
            lw1 = cpool.tile([128, H], BF)
            nc.sync.dma_start(out=lw1[:], in_=lin_wT[128:256, :])
            lwc0 = cpool.tile([128, H], BF)
            nc.sync.dma_start(out=lwc0[:], in_=lwc_in[0:128, :])
            lwc1 = cpool.tile([128, H], BF)
            nc.sync.dma_start(out=lwc1[:], in_=lwc_in[128:256, :])
            dic = cpool.tile([128, NW], F32)
            nc.sync.dma_start(out=dic[:], in_=discols[:])
            dvc = cpool.tile([128, NW], F32)
            nc.sync.dma_start(out=dvc[:], in_=dinvcols[:])
            tl_sb = []
            idx_sb = []
            for s in range(NSUP):
                t = cpool.tile([128, int(NBLK[s])], BF, name=f"tl{s}")
                nc.sync.dma_start(out=t[:], in_=tloc_t[s][:])
                tl_sb.append(t)
                t2 = cpool.tile([128, int(NBLK[s]) * 8], I16, name=f"ix{s}")
                nc.sync.dma_start(out=t2[:], in_=idx_t[s][:])
                idx_sb.append(t2)

            h0 = spool.tile([128, PADN], BF, tag="h0")
            hA = spool.tile([128, PADN], BF, tag="hA")
            hB = spool.tile([128, PADN], BF, tag="hB")
            xws0 = spool.tile([128, PADN], BF, tag="xws0")
            xws1 = spool.tile([128, PADN], BF, tag="xws1")
            tpbank = psA.tile([128, 1024], BF, tag="ps128t", bufs=1)

            ctxs = {}

            qctr = [0]

            def get_ctx(s_step):
                if s_step not in ctxs:
                    tbA = dpool.tile(
                        [TBL_ROWS[0], H], BF, tag=f"tb{s_step}_0", name=f"tbA{s_step}",
                        addr_space="Shared",
                    )
                    tbB = dpool.tile(
                        [TBL_ROWS[1], H], BF, tag=f"tb{s_step}_1", name=f"tbB{s_step}",
                        addr_space="Shared",
                    )
                    ctxs[s_step] = dict(tb=[tbA, tbB], call_tiles={}, w_tiles={}, blkpos=[0] * NSUP)
                return ctxs[s_step]

            def emit_xws(s_step, c):
                """LN (step 1) + h-linear (step 0) + xws for super c's windows."""
                get_ctx(s_step)
                w0c, w1c = SUP_W0[c], SUP_W0[c + 1]
                st = hB if s_step == 1 else h0
                xws = xws1 if s_step == 1 else xws0
                if s_step == 1:
                    # one slab per super: fewer serial cross-engine hops on the
                    # publish critical path; bf16 data doubles DVE throughput
                    g, gw = w0c, w1c - w0c
                    sl = slice(g * 128, (g + gw) * 128)
                    X_t = tpool.tile([128, 25 * 128], BF, tag="ln_X")
                    X = X_t[:, : gw * 128]
                    Y_t = tpool.tile([128, 25 * 128], BF, tag="ln_Y")
                    Y = Y_t[:, : gw * 128]
                    nc.vector.tensor_tensor(out=X, in0=hA[:, sl], in1=h0[:, sl], op=AX.add)
                    mu_t = tpool.tile([128, 25], F32, tag="ln_mu")
                    mu = mu_t[:, :gw]
                    nc.vector.tensor_reduce(out=mu, in_=_r3(X, 128), axis=mybir.AxisListType.X, op=AX.add)
                    nc.vector.tensor_scalar_mul(out=mu, in0=mu, scalar1=1.0 / 128.0)
                    nc.vector.tensor_tensor(out=Y, in0=X, in1=X, op=AX.mult)
                    var_t = tpool.tile([128, 25], F32, tag="ln_var")
                    var = var_t[:, :gw]
                    nc.vector.tensor_reduce(out=var, in_=_r3(Y, 128), axis=mybir.AxisListType.X, op=AX.add)
                    mm_t = tpool.tile([128, 25], F32, tag="ln_mm")
                    mm = mm_t[:, :gw]
                    nc.vector.tensor_tensor(out=mm, in0=mu, in1=mu, op=AX.mult)
                    nc.vector.tensor_scalar(
                        out=var, in0=var, scalar1=1.0 / 128.0, scalar2=LN_EPS, op0=AX.mult, op1=AX.add
                    )
                    nc.vector.tensor_tensor(out=var, in0=var, in1=mm, op=AX.subtract)
                    sd_t = tpool.tile([128, 25], F32, tag="ln_sd")
                    sd = sd_t[:, :gw]
                    nc.scalar.activation(out=sd, in_=var, func=AF.Sqrt)
                    rstd_t = tpool.tile([128, 25], F32, tag="ln_rs")
                    rstd = rstd_t[:, :gw]
                    nc.vector.reciprocal(out=rstd, in_=sd)
                    mb_t = tpool.tile([128, 25], F32, tag="ln_mb")
                    mb = mb_t[:, :gw]
                    nc.vector.tensor_tensor(out=mb, in0=mu, in1=rstd, op=AX.mult)
                    nc.vector.tensor_scalar_mul(out=mb, in0=mb, scalar1=-1.0)
                    for wq in range(gw):
                        nc.scalar.activation(
                            out=X_t[:, wq * 128 : (wq + 1) * 128],
                            in_=X_t[:, wq * 128 : (wq + 1) * 128],
                            func=AF.Identity,
                            scale=rstd_t[:, wq : wq + 1],
                            bias=mb_t[:, wq : wq + 1],
                        )
                    nc.vector.tensor_tensor(out=_r3(Y, 128), in0=_r3(X, 128), in1=_bcast_mid(g1b, gw), op=AX.mult)
                    nc.vector.tensor_tensor(out=_r3(X, 128), in0=_r3(Y, 128), in1=_bcast_mid(b1b, gw), op=AX.add)
                    nc.scalar.activation(out=hB[:, sl], in_=X, func=AF.Relu)
                if s_step == 0:
                    # fused path: xws0 = dis * (ift.T @ (lin_wT @ conv_wT))
                    # (associativity removes the per-window transpose; lin_b is
                    # folded on host and is exactly zero here). ift loads are
                    # batched 4 windows per DMA: sync dispatch is ~0.6us/DMA
                    # and 98 loads were pacing the whole startup.
                    for w4 in range(w0c, w1c, 4):
                        nw4 = min(4, w1c - w4)
                        i0 = ipool.tile([128, 4 * 128], BF, tag="ift")
                        nc.sync.dma_start(
                            out=i0[:, : nw4 * 128], in_=ift[0:128, w4 * 128 : (w4 + nw4) * 128]
                        )
                        i1 = ipool.tile([128, 4 * 128], BF, tag="ift")
                        nc.sync.dma_start(
                            out=i1[:, : nw4 * 128], in_=ift[128:256, w4 * 128 : (w4 + nw4) * 128]
                        )
                        for w in range(w4, w4 + nw4):
                            o = (w - w4) * 128
                            hp = psA.tile([128, 128], F32, tag="ps128")
                            nc.tensor.matmul(hp[:], lhsT=i0[:, o : o + 128], rhs=lw0[:], start=True, stop=False)
                            nc.tensor.matmul(hp[:], lhsT=i1[:, o : o + 128], rhs=lw1[:], start=False, stop=True)
                            nc.vector.tensor_tensor(out=h0[:, ws(w)], in0=hp[:], in1=linb, op=AX.add)
                            xp = psA.tile([128, 128], F32, tag="ps128")
                            nc.tensor.matmul(xp[:], lhsT=i0[:, o : o + 128], rhs=lwc0[:], start=True, stop=False)
                            nc.tensor.matmul(xp[:], lhsT=i1[:, o : o + 128], rhs=lwc1[:], start=False, stop=True)
                            nc.scalar.activation(out=xws[:, ws(w)], in_=xp[:], func=AF.Copy, scale=dic[:, w : w + 1])
                else:
                    # 8-window chunks, three loops per chunk (transposes, PSUM->
                    # SBUF copies, conv+scale): keeps live tiles within the pool
                    # rings while avoiding the per-window PE->DVE->PE ladder of
                    # a fully interleaved emission.
                    for wc in range(w0c, w1c, 8):
                        wn = min(8, w1c - wc)
                        for w in range(wc, wc + wn):
                            sl8 = slice((w - wc) * 128, (w - wc + 1) * 128)
                            nc.tensor.transpose(tpbank[:, sl8], st[:, ws(w)], identb[:])
                        hts = {}
                        for w in range(wc, wc + wn):
                            sl8 = slice((w - wc) * 128, (w - wc + 1) * 128)
                            ht = hpool.tile([128, 128], BF, tag="ht")
                            nc.vector.tensor_copy(out=ht[:], in_=tpbank[:, sl8])
                            hts[w] = ht
                        for w in range(wc, wc + wn):
                            xp = psA.tile([128, 128], F32, tag="ps128")
                            nc.tensor.matmul(xp[:], lhsT=hts[w][:], rhs=cw[:], start=True, stop=True)
                            nc.scalar.activation(
                                out=xws[:, ws(w)], in_=xp[:], func=AF.Copy, scale=dic[:, w : w + 1]
                            )

            def emit_lx_ag(s_step, c):
                """Publish super c's xws to DRAM and AllGather it."""
                ctx = get_ctx(s_step)
                w0c, w1c = SUP_W0[c], SUP_W0[c + 1]
                xws = xws1 if s_step == 1 else xws0
                csz = SUP_SZ[c]
                lx = dpool.tile([csz, H], BF, tag=f"lx{s_step}_{c}", name=f"lx{s_step}_{c}")
                nc.sync.dma_start(
                    out=lx[:].rearrange("(w p) f -> p w f", p=128),
                    in_=_r3(xws[:, w0c * 128 : w1c * 128], 128),
                )
                dst = ctx["tb"][c][:]
                # NB: Trn2 backend requires CollectiveCompute on DMA or Pool
                # engines only — gpsimd (Pool) is the only practical choice.
                nc.gpsimd.collective_compute(
                    "AllGather",
                    AX.bypass,
                    replica_groups=[list(range(NCORES))],
                    ins=[lx.opt()],
                    outs=[dst],
                )

            def call_tile(s_step, s, g, h):
                ctx = ctxs[s_step]
                key = (s, g, h)
                if key not in ctx["call_tiles"]:
                    h0_, h1_, off = halves[(s, g)][h]
                    nb = h1_ - h0_
                    if nb == 0:
                        ctx["call_tiles"][key] = None
                    else:
                        gt = gpool.tile([128, gmax * H], BF, tag="gath")
                        nc.gpsimd.dma_gather(
                            gt[:, : nb * H].rearrange("p (b e) -> p b e", e=H),
                            ctx["tb"][s][:],
                            idx_sb[s][:, off : off + nb * 8],
                            nb * 128,
                            nb * 128,
                            H,
                            single_packet=False,
                            queue_num=qctr[0] % 4,
                        )
                        qctr[0] += 1
                        ctx["call_tiles"][key] = gt
                return ctx["call_tiles"][key]

            def w_tile(s_step, g):
                ctx = ctxs[s_step]
                if g not in ctx["w_tiles"]:
                    wt = wpool.tile([128, wgmax * 128], BF, tag="W")
                    offs = []
                    o = 0
                    for s in range(NSUP):
                        nb = int(gpos[s, g + 1] - gpos[s, g])
                        offs.append(o)
                        # split each build in two: finer DVE ops interleave
                        # better with the latency-critical LN/publish chains
                        for a, b in ((0, nb // 2), (nb // 2, nb)):
                            if b > a:
                                n2 = b - a
                                nc.vector.tensor_tensor(
                                    out=_r3(wt[:, (o + a) * 128 : (o + b) * 128], 128),
                                    in0=tl_sb[s][
                                        :, int(gpos[s, g]) + a : int(gpos[s, g]) + b
                                    ].to_broadcast([128, n2, 128]),
                                    in1=_bcast_mid(iot[:], n2),
                                    op=AX.is_equal,
                                )
                        o += nb
                    ctx["w_tiles"][g] = (wt, offs)
                return ctx["w_tiles"][g]

            def emit_groups(s_step, glo, ghi):
                ctx = ctxs[s_step]
                state = hB if s_step == 1 else h0
                xws_s = xws1 if s_step == 1 else xws0
                hdst = hA
                for grp in range(glo, ghi):
                    bg = grp * 4
                    # Availability-ordered prefetch: super-0 (ready after AG
                    # chunk 1) leads by 4 groups; super-1 (ready after chunk 3)
                    # trails by 2. gpsimd issues in order, so a super-1 call
                    # waiting on the later AG must not block super-0 calls.
                    for gg in (grp, grp + 1, grp + 2, grp + 3):
                        if gg < NGRP:
                            call_tile(s_step, 0, gg, 0)
                            call_tile(s_step, 0, gg, 1)
                    for gg in (grp, grp + 1):
                        if gg < NGRP:
                            call_tile(s_step, 1, gg, 0)
                            call_tile(s_step, 1, gg, 1)
                    gw = min(4, NW - bg)
                    pm = psM.tile([128, 4 * 128], F32, tag="msg")
                    for wq in range(gw):
                        w = bg + wq
                        dst = pm[:, wq * 128 : (wq + 1) * 128]
                        nc.tensor.matmul(dst, lhsT=identb[:], rhs=xws_s[:, ws(w)], start=True, stop=False)
                        nblk = int(B[w].sum())
                        bi = 0
                        for s in range(NSUP):
                            for _ in range(int(B[w, s])):
                                gidx = ctx["blkpos"][s]
                                hh = halves[(s, grp)]
                                h = 0 if gidx < hh[0][1] else 1
                                h0_, h1_, _off = hh[h]
                                ct = call_tile(s_step, s, grp, h)
                                loc = gidx - h0_
                                wt_, woffs = w_tile(s_step, grp)
                                wloc = woffs[s] + (gidx - int(gpos[s, grp]))
                                nc.tensor.matmul(
                                    dst,
                                    lhsT=wt_[:, wloc * 128 : (wloc + 1) * 128],
                                    rhs=ct[:].rearrange("p (b e) -> p b e", e=H)[:, loc, :],
                                    start=False,
                                    stop=(bi == nblk - 1),
                                )
                                ctx["blkpos"][s] += 1
                                bi += 1
                    sl = slice(bg * 128, (bg + gw) * 128)
                    E1_t = tpool.tile([128, 4 * 128], BF, tag="ep_E1")
                    E1 = E1_t[:, : gw * 128]
                    E2_t = tpool.tile([128, 4 * 128], F32, tag="ep_E2")
                    E2 = E2_t[:, : gw * 128]
                    E3_t = tpool.tile([128, 4 * 128], F32, tag="ep_E3")
                    E3 = E3_t[:, : gw * 128]
                    nc.vector.tensor_tensor(
                        out=_r3(E1, 128), in0=_r3(state[:, sl], 128), in1=_bcast_mid(rootb, gw), op=AX.add
                    )
                    for wq in range(gw):
                        w = bg + wq
                        nc.scalar.activation(
                            out=E2_t[:, wq * 128 : (wq + 1) * 128],
                            in_=E1_t[:, wq * 128 : (wq + 1) * 128],
                            func=AF.Relu,
                            scale=dvc[:, w : w + 1],
                        )
                        nc.scalar.activation(
                            out=E3_t[:, wq * 128 : (wq + 1) * 128],
                            in_=pm[:, wq * 128 : (wq + 1) * 128],
                            func=AF.Copy,
                            scale=dic[:, w : w + 1],
                        )
                    nc.vector.tensor_tensor(out=E2, in0=E3, in1=E2, op=AX.add)
                    nc.vector.tensor_tensor(
                        out=_r3(hdst[:, sl], 128), in0=_r3(E2, 128), in1=_bcast_mid(convbr, gw), op=AX.add
                    )
                    if s_step == 1:
                        # stream the finished group out now instead of one big
                        # tail DMA after everything
                        nc.sync.dma_start(
                            out=out_ext[bg * 128 : (bg + gw) * 128, :].rearrange(
                                "(w p) f -> p w f", p=128
                            ),
                            in_=_r3(hdst[:, sl], 128),
                        )

            # software-pipelined emission: step-1 publishes overlap step-0
            # consumption (super-0 = windows 0..23 = groups 0..5). All step-0
            # h-linear/ift work is hoisted ahead of the publishes so super-1's
            # ift loads don't queue behind lx(0,0)'s xws dependency on sync.
            emit_xws(0, 0)
            emit_xws(0, 1)
            emit_lx_ag(0, 0)
            emit_lx_ag(0, 1)
            emit_groups(0, 0, 6)
            emit_xws(1, 0)
            emit_lx_ag(1, 0)
            emit_groups(0, 6, NGRP)
            emit_xws(1, 1)
            emit_lx_ag(1, 1)
            emit_groups(1, 0, NGRP)
    nc.compile()
    return nc


def _rep(v):
    return np.tile(np.asarray(v, np.float32).reshape(1, H), (128, 1))


def kernel_with_results(**inputs):
    in_feat = np.asarray(inputs["in_feat"], np.float32)
    row = np.asarray(inputs["row"]).astype(np.int64)
    col = np.asarray(inputs["col"]).astype(np.int64)
    lin_w = np.asarray(inputs["lin_w"], np.float32)
    lin_b = np.asarray(inputs["lin_b"], np.float32)
    conv_w = np.asarray(inputs["conv_w"], np.float32)
    conv_b = np.asarray(inputs["conv_b"], np.float32)
    root_emb = np.asarray(inputs["root_emb"], np.float32)
    ln_gamma = np.asarray(inputs["ln_gamma"], np.float32)
    ln_beta = np.asarray(inputs["ln_beta"], np.float32)

    g = _prep_graph(row, col)
    nc = _build(g["B"], g["gpos"], g["NBLK"])

    ift_full = np.ascontiguousarray(in_feat.T)
    consts = np.concatenate(
        [_rep(lin_b), _rep(root_emb[0]), _rep(conv_b), _rep(ln_gamma[1]), _rep(ln_beta[1])],
        axis=1,
    )
    iota = np.tile(np.arange(128, dtype=np.float32), (128, 1)).astype(BF16)
    lin_wT = np.ascontiguousarray(lin_w.T).astype(BF16)
    conv_wT = np.ascontiguousarray(conv_w.T).astype(BF16)
    # composed weight for the fused step-0 xws path (lin_b term is zero)
    lwc = np.ascontiguousarray(lin_w.T.astype(np.float64) @ conv_w.T.astype(np.float64)).astype(BF16)
    assert not np.any(lin_b), "fused xws0 path assumes lin_b == 0"

    in_maps = []
    for k in range(NCORES):
        ift_k = np.zeros((IN, PADN), BF16)
        ift_k[:, :NPC] = ift_full[:, k * NPC : (k + 1) * NPC].astype(BF16)
        m = {
            "ift": ift_k,
            "lin_wT": lin_wT,
            "lwc": lwc,
            "conv_wT": conv_wT,
            "consts": consts,
            "iota": iota,
            "discols": g["dis_cols"][k],
            "dinvcols": g["dinv_cols"][k],
        }
        for s in range(NSUP):
            m[f"idx{s}"] = g["idx_w"][s][k]
            m[f"tloc{s}"] = np.ascontiguousarray(g["tlocs"][s][k])
        in_maps.append(m)

    res = run_bass_kernel_spmd(nc, in_maps, list(range(NCORES)))
    out = np.concatenate(
        [np.asarray(res.results[k]["out"])[:NPC] for k in range(NCORES)], axis=0
    )
    return out.astype(np.float32), res


def kernel(**inputs):
    out, _ = kernel_with_results(**inputs)
    return out



# revision 45
# speedup vs baseline: 1.1165x; 1.1165x over previous
"""EnhancedGCN on 8 Trainium2 NeuronCores (Bass/Tile, SPMD).

Strategy: 1D node partition (6250 nodes/core, padded to 6272). Small weights
replicated. Per propagation step: each core computes xws = dis * (h @ conv_w.T)
for its nodes, AllGathers the bf16 table (4 collectives writing slices of two
int16-addressable super-tables, pipelined behind the xws windows), then
gathers source rows per edge (dma_gather over 4 SWDGE queues), reduces them
into per-target sums with 0/1 selection-matrix matmuls accumulating in PSUM
(self-loops enter as an identity-matmul block), and applies the pointwise
epilogue (degree norm, root/relu term, residual+LN between steps).
Edge weights ew = dis[t]*dis[s] are separable: they fold into a pre-scale of
the table (dis[s]) and a post-scale of the message sum (dis[t]).
Host-side work is limited to graph-structure prep (sorting edges into
target windows, block padding, int16 index streams) and weight transposes.
"""
import sys

sys.path.insert(0, "/opt/trn_rl_repo")

import numpy as np
import ml_dtypes

import concourse.bass as bass
import concourse.bacc as bacc
import concourse.tile as tile
import concourse.mybir as mybir
from concourse.bass_utils import run_bass_kernel_spmd
from concourse.masks import make_identity

BF16 = ml_dtypes.bfloat16
N, IN, H = 50000, 256, 128
NCORES = 8
NPC = N // NCORES  # 6250
NW = (NPC + 127) // 128  # 49
PADN = NW * 128  # 6272
LN_EPS = 1e-5
NGRP = (NW + 3) // 4  # 13 groups of 4 windows

# Two gather super-streams, one AllGather each (Shared-output collectives
# require a single writing instruction per Shared tensor).
NSUP = 2
SUP_W = [24, 25]  # windows per super
SUP_W0 = [0, 24, 49]
SUP_SZ = [w * 128 for w in SUP_W]
# table row layout per super: [8 ranks x sup_sz]
TBL_ROWS = [8 * SUP_SZ[0], 8 * SUP_SZ[1]]

F32 = mybir.dt.float32
BF = mybir.dt.bfloat16
I16 = mybir.dt.int16
AX = mybir.AluOpType
AF = mybir.ActivationFunctionType


def _bcast_mid(ap, n):
    """[128, F] AP -> [128, n, F] with stride-0 middle dim."""
    a = ap.copy()
    a.ap = [a.ap[0], [0, n]] + a.ap[1:]
    return a


def _r3(ap, f):
    return ap.rearrange("p (w f) -> p w f", f=f)


def _wrap_idx(idx):
    """flat idx [n] (n % 16 == 0) -> [128, n/16] int16 wrapped + replicated."""
    n = len(idx)
    t = idx.reshape(n // 16, 16).T.astype(np.int16)
    return np.tile(t, (8, 1))


def _prep_graph(row, col):
    """Graph-structure-only preprocessing (row/col ints)."""
    deg = np.bincount(row, minlength=N).astype(np.float64) + 1.0
    dis_f = 1.0 / np.sqrt(deg)
    dinv_f = 1.0 / deg

    core = row // NPC
    src_core = col // NPC
    src_off = col % NPC
    src_w = src_off >> 7
    src_sup = (src_w >= SUP_W0[1]).astype(np.int64)
    csz = np.asarray(SUP_SZ)[src_sup]
    w0 = np.asarray(SUP_W0)[src_sup] * 128
    src_idx = src_core * csz + (src_off - w0)

    per_core = []
    counts = np.zeros((NCORES, NW, NSUP), np.int64)
    for k in range(NCORES):
        m = core == k
        tgt = (row[m] - k * NPC).astype(np.int64)
        sidx = src_idx[m]
        ssup = src_sup[m]
        w = tgt >> 7
        order = np.argsort(w, kind="stable")
        tgt, sidx, ssup, w = tgt[order], sidx[order], ssup[order], w[order]
        ents = []
        bounds = np.searchsorted(w, np.arange(NW + 1))
        for wi in range(NW):
            sl = slice(bounds[wi], bounds[wi + 1])
            s_w, t_w, u_w = sidx[sl], tgt[sl] - (wi << 7), ssup[sl]
            by_sup = []
            for s in range(NSUP):
                mm = u_w == s
                # NB: do NOT sort by source here — sorted in-flight gather
                # descriptors concentrate on few DRAM channels and measured
                # 1.7x slower than the random edge order.
                by_sup.append((s_w[mm], t_w[mm]))
                counts[k, wi, s] = int(mm.sum())
            ents.append(by_sup)
        per_core.append(ents)

    B = np.ceil(counts.max(axis=0) / 128).astype(np.int64)  # [NW, NSUP]
    NBLK = B.sum(axis=0).astype(np.int64)  # per super
    gpos = np.zeros((NSUP, NGRP + 1), np.int64)
    for s in range(NSUP):
        pref = np.concatenate([[0], np.cumsum(B[:, s])])
        for g in range(NGRP + 1):
            gpos[s, g] = pref[min(g * 4, NW)]

    # padding descriptors read random rows (masked by -1 tlocs): random spreads
    # the dead reads across DRAM channels like the real traffic
    rng = np.random.default_rng(12345)
    idx_streams = [np.empty((NCORES, int(NBLK[s]) * 128), np.int64) for s in range(NSUP)]
    for s in range(NSUP):
        idx_streams[s][:] = rng.integers(0, TBL_ROWS[s], idx_streams[s].shape)
    tlocs = [np.full((NCORES, 128, int(NBLK[s])), -1.0, np.float32) for s in range(NSUP)]

    for k in range(NCORES):
        pos = [0] * NSUP
        for wi in range(NW):
            for s in range(NSUP):
                s_w, t_w = per_core[k][wi][s]
                n = len(s_w)
                p = pos[s]
                idx_streams[s][k, p * 128 : p * 128 + n] = s_w
                j = np.arange(n)
                tlocs[s][k, j % 128, p + j // 128] = t_w
                pos[s] += int(B[wi, s])

    # wrap idx per half-call segment (aligned to group boundaries, split in two)
    idx_w = [None] * NSUP
    for s in range(NSUP):
        per_core_w = [[] for _ in range(NCORES)]
        for g in range(NGRP):
            b0, b1 = int(gpos[s, g]), int(gpos[s, g + 1])
            mid = b0 + (b1 - b0 + 1) // 2
            for (h0_, h1_) in ((b0, mid), (mid, b1)):
                if h1_ > h0_:
                    for k in range(NCORES):
                        per_core_w[k].append(
                            _wrap_idx(idx_streams[s][k, h0_ * 128 : h1_ * 128])
                        )
        idx_w[s] = np.stack([np.concatenate(x, axis=1) for x in per_core_w])

    dis_cols = np.zeros((NCORES, 128, NW), np.float32)
    dinv_cols = np.ones((NCORES, 128, NW), np.float32)
    for k in range(NCORES):
        v = np.zeros(PADN, np.float64)
        v[:NPC] = dis_f[k * NPC : (k + 1) * NPC]
        dis_cols[k] = v.reshape(NW, 128).T
        u = np.ones(PADN, np.float64)
        u[:NPC] = dinv_f[k * NPC : (k + 1) * NPC]
        dinv_cols[k] = u.reshape(NW, 128).T

    return dict(
        B=B,
        NBLK=NBLK,
        gpos=gpos,
        idx_w=idx_w,
        tlocs=[t.astype(BF16) for t in tlocs],
        dis_cols=dis_cols,
        dinv_cols=dinv_cols,
    )


def _build(B, gpos, NBLK):
    nc = bacc.Bacc("TRN2", target_bir_lowering=False, debug=False, num_swdge_queues=4)

    ift = nc.dram_tensor("ift", [IN, PADN], BF, kind="ExternalInput")
    lin_wT = nc.dram_tensor("lin_wT", [IN, H], BF, kind="ExternalInput")
    conv_wT = nc.dram_tensor("conv_wT", [H, H], BF, kind="ExternalInput")
    consts = nc.dram_tensor("consts", [128, 5 * H], F32, kind="ExternalInput")
    iota_in = nc.dram_tensor("iota", [128, 128], BF, kind="ExternalInput")
    discols = nc.dram_tensor("discols", [128, NW], F32, kind="ExternalInput")
    dinvcols = nc.dram_tensor("dinvcols", [128, NW], F32, kind="ExternalInput")
    idx_t = [
        nc.dram_tensor(f"idx{s}", [128, int(NBLK[s]) * 8], I16, kind="ExternalInput")
        for s in range(NSUP)
    ]
    tloc_t = [
        nc.dram_tensor(f"tloc{s}", [128, int(NBLK[s])], BF, kind="ExternalInput")
        for s in range(NSUP)
    ]
    out_ext = nc.dram_tensor("out", [PADN, H], BF, kind="ExternalOutput")

    def ws(w):
        return slice(w * 128, (w + 1) * 128)

    # per-(stream, group) half-call boundaries + column offset into wrapped idx
    halves = {}
    for s in range(NSUP):
        off = 0
        for g in range(NGRP):
            b0, b1 = int(gpos[s, g]), int(gpos[s, g + 1])
            mid = b0 + (b1 - b0 + 1) // 2
            hs = []
            for (h0_, h1_) in ((b0, mid), (mid, b1)):
                hs.append((h0_, h1_, off))
                off += (h1_ - h0_) * 8
            halves[(s, g)] = hs
    gmax = max(h1 - h0 for v in halves.values() for (h0, h1, _) in v)
    wgmax = int(
        max(sum(int(gpos[s, g + 1] - gpos[s, g]) for s in range(NSUP)) for g in range(NGRP))
    )

    with tile.TileContext(nc) as tc:
        with (
            tc.tile_pool(name="const", bufs=1) as cpool,
            tc.tile_pool(name="state", bufs=1) as spool,
            tc.tile_pool(name="iftp", bufs=12) as ipool,
            tc.tile_pool(name="gath", bufs=14) as gpool,
            tc.tile_pool(name="wp", bufs=3) as wpool,
            tc.tile_pool(name="tmp", bufs=1) as tpool,
            tc.tile_pool(name="psA", bufs=2, space="PSUM") as psA,
            tc.tile_pool(name="psT", bufs=1, space="PSUM") as psT,
            tc.tile_pool(name="psM", bufs=3, space="PSUM") as psM,
            tc.tile_pool(name="psM2", bufs=2, space="PSUM") as psM2,
            tc.tile_pool(name="dram", bufs=1, space="DRAM") as dpool,
        ):
            identf = cpool.tile([128, 128], F32)
            make_identity(nc, identf[:])
            identb = cpool.tile([128, 128], BF)
            nc.vector.tensor_copy(out=identb[:], in_=identf[:])
            cst = cpool.tile([128, 5 * H], F32)
            nc.sync.dma_start(out=cst[:], in_=consts[:])
            linb, rootr, convbr, g1r, b1r = (cst[:, i * H : (i + 1) * H] for i in range(5))
            # bf16 copies for ops whose data operands are bf16 (avoid
            # mixed-input-dtype DVE ops)
            cstb = cpool.tile([128, 5 * H], BF)
            nc.vector.tensor_copy(out=cstb[:], in_=cst[:])
            _, rootb, _, g1b, b1b = (cstb[:, i * H : (i + 1) * H] for i in range(5))
            iot = cpool.tile([128, 128], BF)
            nc.sync.dma_start(out=iot[:], in_=iota_in[:])
            cw = cpool.tile([128, H], BF)
            nc.sync.dma_start(out=cw[:], in_=conv_wT[:])
            lw0 = cpool.tile([128, H], BF)
            nc.sync.dma_start(out=lw0[:], in_=lin_wT[0:128, :])
            lw1 = cpool.tile([128, H], BF)
            nc.sync.dma_start(out=lw1[:], in_=lin_wT[128:256, :])
            dic = cpool.tile([128, NW], F32)
            nc.sync.dma_start(out=dic[:], in_=discols[:])
            dvc = cpool.tile([128, NW], F32)
            nc.sync.dma_start(out=dvc[:], in_=dinvcols[:])
            dicb = cpool.tile([128, NW], BF)
            nc.vector.tensor_copy(out=dicb[:], in_=dic[:])
            tl_sb = []
            idx_sb = []
            for s in range(NSUP):
                t = cpool.tile([128, int(NBLK[s])], BF, name=f"tl{s}")
                nc.sync.dma_start(out=t[:], in_=tloc_t[s][:])
                tl_sb.append(t)
                t2 = cpool.tile([128, int(NBLK[s]) * 8], I16, name=f"ix{s}")
                nc.sync.dma_start(out=t2[:], in_=idx_t[s][:])
                idx_sb.append(t2)

            h0 = spool.tile([128, PADN], BF, tag="h0")
            hA = spool.tile([128, PADN], BF, tag="hA")
            hB = spool.tile([128, PADN], BF, tag="hB")
            xws0 = spool.tile([128, PADN], BF, tag="xws0")
            xws1 = spool.tile([128, PADN], BF, tag="xws1")
            tp32 = psT.tile([128, 512], F32, tag="tp32")

            ctxs = {}

            qctr = [0]

            def get_ctx(s_step):
                if s_step not in ctxs:
                    tbA = dpool.tile(
                        [TBL_ROWS[0], H], BF, tag=f"tb{s_step}_0", name=f"tbA{s_step}",
                        addr_space="Shared",
                    )
                    tbB = dpool.tile(
                        [TBL_ROWS[1], H], BF, tag=f"tb{s_step}_1", name=f"tbB{s_step}",
                        addr_space="Shared",
                    )
                    ctxs[s_step] = dict(tb=[tbA, tbB], call_tiles={}, w_tiles={}, blkpos=[0] * NSUP)
                return ctxs[s_step]

            def emit_xws(s_step, c):
                """LN (step 1) + h-linear (step 0) + xws for super c's windows."""
                get_ctx(s_step)
                w0c, w1c = SUP_W0[c], SUP_W0[c + 1]
                st = hB if s_step == 1 else h0
                xws = xws1 if s_step == 1 else xws0
                if s_step == 1:
                    # one slab per super: fewer serial cross-engine hops on the
                    # publish critical path; bf16 data doubles DVE throughput
                    g, gw = w0c, w1c - w0c
                    sl = slice(g * 128, (g + gw) * 128)
                    X_t = tpool.tile([128, 25 * 128], BF, tag="ln_X")
                    X = X_t[:, : gw * 128]
                    Y_t = tpool.tile([128, 25 * 128], BF, tag="ln_Y")
                    Y = Y_t[:, : gw * 128]
                    nc.vector.tensor_tensor(out=X, in0=hA[:, sl], in1=h0[:, sl], op=AX.add)
                    mu_t = tpool.tile([128, 25], F32, tag="ln_mu")
                    mu = mu_t[:, :gw]
                    nc.vector.tensor_reduce(out=mu, in_=_r3(X, 128), axis=mybir.AxisListType.X, op=AX.add)
                    nc.vector.tensor_scalar_mul(out=mu, in0=mu, scalar1=1.0 / 128.0)
                    nc.vector.tensor_tensor(out=Y, in0=X, in1=X, op=AX.mult)
                    var_t = tpool.tile([128, 25], F32, tag="ln_var")
                    var = var_t[:, :gw]
                    nc.vector.tensor_reduce(out=var, in_=_r3(Y, 128), axis=mybir.AxisListType.X, op=AX.add)
                    mm_t = tpool.tile([128, 25], F32, tag="ln_mm")
                    mm = mm_t[:, :gw]
                    nc.vector.tensor_tensor(out=mm, in0=mu, in1=mu, op=AX.mult)
                    nc.vector.tensor_scalar(
                        out=var, in0=var, scalar1=1.0 / 128.0, scalar2=LN_EPS, op0=AX.mult, op1=AX.add
                    )
                    nc.vector.tensor_tensor(out=var, in0=var, in1=mm, op=AX.subtract)
                    sd_t = tpool.tile([128, 25], F32, tag="ln_sd")
                    sd = sd_t[:, :gw]
                    nc.scalar.activation(out=sd, in_=var, func=AF.Sqrt)
                    rstd_t = tpool.tile([128, 25], F32, tag="ln_rs")
                    rstd = rstd_t[:, :gw]
                    nc.vector.reciprocal(out=rstd, in_=sd)
                    mb_t = tpool.tile([128, 25], F32, tag="ln_mb")
                    mb = mb_t[:, :gw]
                    nc.vector.tensor_tensor(out=mb, in0=mu, in1=rstd, op=AX.mult)
                    nc.vector.tensor_scalar_mul(out=mb, in0=mb, scalar1=-1.0)
                    # normalize via two broadcast DVE ops instead of 25 serial
                    # per-window ACT calls (publish-chain latency)
                    rstdb_t = tpool.tile([128, 25], BF, tag="ln_rsb")
                    rstdb = rstdb_t[:, :gw]
                    nc.vector.tensor_copy(out=rstdb, in_=rstd)
                    mbb_t = tpool.tile([128, 25], BF, tag="ln_mbb")
                    mbb = mbb_t[:, :gw]
                    nc.vector.tensor_copy(out=mbb, in_=mb)
                    nc.vector.tensor_tensor(
                        out=_r3(Y, 128), in0=_r3(X, 128), in1=rstdb.to_broadcast([128, gw, 128]), op=AX.mult
                    )
                    nc.vector.tensor_tensor(
                        out=_r3(X, 128), in0=_r3(Y, 128), in1=mbb.to_broadcast([128, gw, 128]), op=AX.add
                    )
                    nc.vector.tensor_tensor(out=_r3(Y, 128), in0=_r3(X, 128), in1=_bcast_mid(g1b, gw), op=AX.mult)
                    nc.vector.tensor_tensor(out=_r3(X, 128), in0=_r3(Y, 128), in1=_bcast_mid(b1b, gw), op=AX.add)
                    nc.scalar.activation(out=hB[:, sl], in_=X, func=AF.Relu)
                if s_step == 0:
                    # ift loads batched 4 windows per DMA: sync dispatch is
                    # ~0.6us/DMA and 98 loads were pacing the whole startup
                    for w4 in range(w0c, w1c, 4):
                        nw4 = min(4, w1c - w4)
                        i0 = ipool.tile([128, 4 * 128], BF, tag="ift")
                        nc.sync.dma_start(
                            out=i0[:, : nw4 * 128], in_=ift[0:128, w4 * 128 : (w4 + nw4) * 128]
                        )
                        i1 = ipool.tile([128, 4 * 128], BF, tag="ift")
                        nc.sync.dma_start(
                            out=i1[:, : nw4 * 128], in_=ift[128:256, w4 * 128 : (w4 + nw4) * 128]
                        )
                        for w in range(w4, w4 + nw4):
                            o = (w - w4) * 128
                            hp = psA.tile([128, 128], F32, tag="ps128")
                            nc.tensor.matmul(hp[:], lhsT=i0[:, o : o + 128], rhs=lw0[:], start=True, stop=False)
                            nc.tensor.matmul(hp[:], lhsT=i1[:, o : o + 128], rhs=lw1[:], start=False, stop=True)
                            nc.vector.tensor_tensor(out=h0[:, ws(w)], in0=hp[:], in1=linb, op=AX.add)
                # the table is h-space (dis * h): segment_sum commutes with the
                # conv linear map, so conv_w is applied on the consume side to
                # the aggregated sums. This removes every transpose and matmul
                # from the publish critical path; one broadcast DVE op scales.
                gw = w1c - w0c
                nc.vector.tensor_tensor(
                    out=_r3(xws[:, w0c * 128 : w1c * 128], 128),
                    in0=_r3(st[:, w0c * 128 : w1c * 128], 128),
                    in1=dicb[:, w0c:w1c].to_broadcast([128, gw, 128]),
                    op=AX.mult,
                )

            def emit_lx_ag(s_step, c):
                """Publish super c's xws to DRAM and AllGather it."""
                ctx = get_ctx(s_step)
                w0c, w1c = SUP_W0[c], SUP_W0[c + 1]
                xws = xws1 if s_step == 1 else xws0
                csz = SUP_SZ[c]
                lx = dpool.tile([csz, H], BF, tag=f"lx{s_step}_{c}", name=f"lx{s_step}_{c}")
                nc.sync.dma_start(
                    out=lx[:].rearrange("(w p) f -> p w f", p=128),
                    in_=_r3(xws[:, w0c * 128 : w1c * 128], 128),
                )
                dst = ctx["tb"][c][:]
                # NB: Trn2 backend requires CollectiveCompute on DMA or Pool
                # engines only — gpsimd (Pool) is the only practical choice.
                nc.gpsimd.collective_compute(
                    "AllGather",
                    AX.bypass,
                    replica_groups=[list(range(NCORES))],
                    ins=[lx.opt()],
                    outs=[dst],
                )

            def call_tile(s_step, s, g, h):
                ctx = ctxs[s_step]
                key = (s, g, h)
                if key not in ctx["call_tiles"]:
                    h0_, h1_, off = halves[(s, g)][h]
                    nb = h1_ - h0_
                    if nb == 0:
                        ctx["call_tiles"][key] = None
                    else:
                        gt = gpool.tile([128, gmax * H], BF, tag="gath")
                        nc.gpsimd.dma_gather(
                            gt[:, : nb * H].rearrange("p (b e) -> p b e", e=H),
                            ctx["tb"][s][:],
                            idx_sb[s][:, off : off + nb * 8],
                            nb * 128,
                            nb * 128,
                            H,
                            single_packet=False,
                            queue_num=qctr[0] % 4,
                        )
                        qctr[0] += 1
                        ctx["call_tiles"][key] = gt
                return ctx["call_tiles"][key]

            def w_tile(s_step, g):
                ctx = ctxs[s_step]
                if g not in ctx["w_tiles"]:
                    wt = wpool.tile([128, wgmax * 128], BF, tag="W")
                    offs = []
                    o = 0
                    for s in range(NSUP):
                        nb = int(gpos[s, g + 1] - gpos[s, g])
                        offs.append(o)
                        if nb > 0:
                            nc.vector.tensor_tensor(
                                out=_r3(wt[:, o * 128 : (o + nb) * 128], 128),
                                in0=tl_sb[s][:, int(gpos[s, g]) : int(gpos[s, g + 1])].to_broadcast([128, nb, 128]),
                                in1=_bcast_mid(iot[:], nb),
                                op=AX.is_equal,
                            )
                        o += nb
                    ctx["w_tiles"][g] = (wt, offs)
                return ctx["w_tiles"][g]

            def emit_groups(s_step, glo, ghi):
                ctx = ctxs[s_step]
                state = hB if s_step == 1 else h0
                xws_s = xws1 if s_step == 1 else xws0
                hdst = hA
                for grp in range(glo, ghi):
                    bg = grp * 4
                    # Availability-ordered prefetch: super-0 (ready after AG
                    # chunk 1) leads by 4 groups; super-1 (ready after chunk 3)
                    # trails by 2. gpsimd issues in order, so a super-1 call
                    # waiting on the later AG must not block super-0 calls.
                    for gg in (grp, grp + 1, grp + 2, grp + 3):
                        if gg < NGRP:
                            call_tile(s_step, 0, gg, 0)
                            call_tile(s_step, 0, gg, 1)
                    for gg in (grp, grp + 1):
                        if gg < NGRP:
                            call_tile(s_step, 1, gg, 0)
                            call_tile(s_step, 1, gg, 1)
                    gw = min(4, NW - bg)
                    pm = psM.tile([128, 4 * 128], F32, tag="msg")
                    for wq in range(gw):
                        w = bg + wq
                        dst = pm[:, wq * 128 : (wq + 1) * 128]
                        nc.tensor.matmul(dst, lhsT=identb[:], rhs=xws_s[:, ws(w)], start=True, stop=False)
                        nblk = int(B[w].sum())
                        bi = 0
                        for s in range(NSUP):
                            for _ in range(int(B[w, s])):
                                gidx = ctx["blkpos"][s]
                                hh = halves[(s, grp)]
                                h = 0 if gidx < hh[0][1] else 1
                                h0_, h1_, _off = hh[h]
                                ct = call_tile(s_step, s, grp, h)
                                loc = gidx - h0_
                                wt_, woffs = w_tile(s_step, grp)
                                wloc = woffs[s] + (gidx - int(gpos[s, grp]))
                                nc.tensor.matmul(
                                    dst,
                                    lhsT=wt_[:, wloc * 128 : (wloc + 1) * 128],
                                    rhs=ct[:].rearrange("p (b e) -> p b e", e=H)[:, loc, :],
                                    start=False,
                                    stop=(bi == nblk - 1),
                                )
                                ctx["blkpos"][s] += 1
                                bi += 1
                    sl = slice(bg * 128, (bg + gw) * 128)
                    E1_t = tpool.tile([128, 4 * 128], BF, tag="ep_E1")
                    E1 = E1_t[:, : gw * 128]
                    E2_t = tpool.tile([128, 4 * 128], F32, tag="ep_E2")
                    E2 = E2_t[:, : gw * 128]
                    E3_t = tpool.tile([128, 4 * 128], F32, tag="ep_E3")
                    e3b = tpool.tile([128, 4 * 128], BF, tag="ep_e3b")
                    pm2 = psM2.tile([128, 4 * 128], F32, tag="msg2")
                    nc.vector.tensor_tensor(
                        out=_r3(E1, 128), in0=_r3(state[:, sl], 128), in1=_bcast_mid(rootb, gw), op=AX.add
                    )
                    for wq in range(gw):
                        w = bg + wq
                        w128 = slice(wq * 128, (wq + 1) * 128)
                        nc.scalar.activation(
                            out=E2_t[:, w128],
                            in_=E1_t[:, w128],
                            func=AF.Relu,
                            scale=dvc[:, w : w + 1],
                        )
                        # consume-side conv: E3 = dic * (sum ew h), transpose to
                        # put features on partitions, then apply conv_wT
                        nc.scalar.activation(
                            out=E3_t[:, w128], in_=pm[:, w128], func=AF.Copy, scale=dic[:, w : w + 1]
                        )
                        nc.tensor.transpose(tp32[:, w128], E3_t[:, w128], identf[:])
                        nc.scalar.copy(out=e3b[:, w128], in_=tp32[:, w128])
                        nc.tensor.matmul(pm2[:, w128], lhsT=e3b[:, w128], rhs=cw[:], start=True, stop=True)
                    nc.vector.tensor_tensor(out=E2, in0=pm2[:, : gw * 128], in1=E2, op=AX.add)
                    nc.vector.tensor_tensor(
                        out=_r3(hdst[:, sl], 128), in0=_r3(E2, 128), in1=_bcast_mid(convbr, gw), op=AX.add
                    )
                    if s_step == 1:
                        # stream the finished group out now instead of one big
                        # tail DMA after everything
                        nc.sync.dma_start(
                            out=out_ext[bg * 128 : (bg + gw) * 128, :].rearrange(
                                "(w p) f -> p w f", p=128
                            ),
                            in_=_r3(hdst[:, sl], 128),
                        )

            # software-pipelined emission: step-1 publishes overlap step-0
            # consumption (super-0 = windows 0..23 = groups 0..5). All step-0
            # h-linear/ift work is hoisted ahead of the publishes so super-1's
            # ift loads don't queue behind lx(0,0)'s xws dependency on sync.
            emit_xws(0, 0)
            emit_xws(0, 1)
            emit_lx_ag(0, 0)
            emit_lx_ag(0, 1)
            emit_groups(0, 0, 6)
            emit_xws(1, 0)
            emit_lx_ag(1, 0)
            emit_groups(0, 6, NGRP)
            emit_xws(1, 1)
            emit_lx_ag(1, 1)
            emit_groups(1, 0, NGRP)
    nc.compile()
    return nc


def _rep(v):
    return np.tile(np.asarray(v, np.float32).reshape(1, H), (128, 1))


def kernel_with_results(**inputs):
    in_feat = np.asarray(inputs["in_feat"], np.float32)
    row = np.asarray(inputs["row"]).astype(np.int64)
    col = np.asarray(inputs["col"]).astype(np.int64)
    lin_w = np.asarray(inputs["lin_w"], np.float32)
    lin_b = np.asarray(inputs["lin_b"], np.float32)
    conv_w = np.asarray(inputs["conv_w"], np.float32)
    conv_b = np.asarray(inputs["conv_b"], np.float32)
    root_emb = np.asarray(inputs["root_emb"], np.float32)
    ln_gamma = np.asarray(inputs["ln_gamma"], np.float32)
    ln_beta = np.asarray(inputs["ln_beta"], np.float32)

    g = _prep_graph(row, col)
    nc = _build(g["B"], g["gpos"], g["NBLK"])

    ift_full = np.ascontiguousarray(in_feat.T)
    consts = np.concatenate(
        [_rep(lin_b), _rep(root_emb[0]), _rep(conv_b), _rep(ln_gamma[1]), _rep(ln_beta[1])],
        axis=1,
    )
    iota = np.tile(np.arange(128, dtype=np.float32), (128, 1)).astype(BF16)
    lin_wT = np.ascontiguousarray(lin_w.T).astype(BF16)
    conv_wT = np.ascontiguousarray(conv_w.T).astype(BF16)

    in_maps = []
    for k in range(NCORES):
        ift_k = np.zeros((IN, PADN), BF16)
        ift_k[:, :NPC] = ift_full[:, k * NPC : (k + 1) * NPC].astype(BF16)
        m = {
            "ift": ift_k,
            "lin_wT": lin_wT,
            "conv_wT": conv_wT,
            "consts": consts,
            "iota": iota,
            "discols": g["dis_cols"][k],
            "dinvcols": g["dinv_cols"][k],
        }
        for s in range(NSUP):
            m[f"idx{s}"] = g["idx_w"][s][k]
            m[f"tloc{s}"] = np.ascontiguousarray(g["tlocs"][s][k])
        in_maps.append(m)

    res = run_bass_kernel_spmd(nc, in_maps, list(range(NCORES)))
    out = np.concatenate(
        [np.asarray(res.results[k]["out"])[:NPC] for k in range(NCORES)], axis=0
    )
    return out.astype(np.float32), res


def kernel(**inputs):
    out, _ = kernel_with_results(**inputs)
    return out

